# revision 10
# baseline (speedup 1.0000x reference)
"""Trainium2 Bass kernel for nn_BispectrumPool.

Math (validated vs reference):
  F = FFT_8 along the group axis. beta[k] = F1*F[k]*conj(F[1+k mod 8]).
  Due to conjugate symmetry of the real-input FFT:
    beta4=beta3, beta5=beta2, beta6=beta1, beta7=beta0 (real), Im(beta0)=0
  -> only 7 distinct nonzero features per channel:
     [beta0r, beta1r, beta1i, beta2r, beta2i, beta3r, beta3i]
  with
     beta0r = F0*(b1^2+b2^2)
     beta1  = G*conj(F2),  G = F1^2      (Gr=b1^2-b2^2, Gi=2*b1b2)
     beta2  = F2*H,        H = F1*conj(F3)
     beta3  = F4*K,        K = F1*F3
  (b1,b2)=(Re,Im)F1, (b3,b4)=F2, (b5,b6)=F3, b7=F4(real), b0=F0(real).
  feat = ln(1+relu(beta_part)); y = W_folded @ feat + bias, where the 16
  original feature columns fold onto the 7 distinct ones (cols 8,15 drop).

Distribution: pure data parallel, batch 16 -> 2 per core on 8 cores.

Per core pipeline, per (b, 448-col chunk, 16-channel block q):
  DMA x (bf16, one DMA per (b,chunk) covering all 4 q-blocks)
  PE  : U,V,A form matmuls (block-diag DFT rows, bf16 in / f32 PSUM);
        C1a/C1b combine matmuls; Rpre combine matmuls
  Pool: vsb/asb PSUM->SBUF evacuations; t2 product
  DVE : m1 = U*V (7 products); t1 = A*C1a (8 products)
  ACT : relu; ln(1+x); conv bias add
  PE  : 4 conv matmuls (contraction 4x112) -> y [64, 448] -> ybig SBUF
  DMA y once per batch row
"""

import numpy as np

C, G = 64, 8
HWP = 56 * 56            # 3136
S = 448                  # chunk width (3136 = 7*448)
NCHUNK = HWP // S        # 7
NCORES = 8
BPC = 2                  # batches per core
NQ = 4                   # channel blocks of 16


def _form_rows():
    g = np.arange(G)
    B1 = np.cos(2 * np.pi * g / G)
    B2 = -np.sin(2 * np.pi * g / G)
    B3 = np.cos(4 * np.pi * g / G)
    B4 = -np.sin(4 * np.pi * g / G)
    B5 = np.cos(6 * np.pi * g / G)
    B6 = -np.sin(6 * np.pi * g / G)
    B7 = np.cos(np.pi * g)
    B0 = np.ones(G)
    U = np.stack([B1, B2, B1, B1, B2, B2, B1])            # 7 rows
    V = np.stack([B1, B2, B2, B5, B6, B5, B6])            # 7 rows
    # blocks 4..6 = [b3, b4, b7] so the T2 product can slice at partition 64
    # (SBUF engine access must start at a 32-aligned partition)
    A = np.stack([B0, B3, B4, B3, B3, B4, B7, B4])        # 8 rows
    return U, V, A


def _combine_mats():
    # M1 blocks: [b1^2, b2^2, b1b2, b1b5, b2b6, b2b5, b1b6]
    Wc_a = np.zeros((8, 7))
    Wc_a[0, 0] = Wc_a[0, 1] = 1              # S+
    Wc_a[1, 0], Wc_a[1, 1] = 1, -1           # Gr
    Wc_a[2, 2] = 2.0                         # Gi
    Wc_a[3, 2] = 2.0                         # Gi
    Wc_a[4, 3] = Wc_a[4, 4] = 1              # Hr
    Wc_a[5, 5], Wc_a[5, 6] = 1, -1           # Hi
    Wc_a[6, 3], Wc_a[6, 4] = 1, -1           # Kr
    Wc_a[7, 0], Wc_a[7, 1] = 1, -1           # Gr
    Wc_b = np.zeros((3, 7))
    Wc_b[0, 5], Wc_b[0, 6] = 1, -1           # Hi
    Wc_b[1, 3] = Wc_b[1, 4] = 1              # Hr
    Wc_b[2, 5] = Wc_b[2, 6] = 1              # Ki
    # T1 blocks: [b0S+, b3Gr, b4Gi, b3Gi, b3Hr, b4Hi, b7Kr, b4Gr]
    # T2 blocks: [b3Hi, b4Hr, b7Ki]
    Wr_1 = np.zeros((7, 8))
    Wr_1[0, 0] = 1                            # beta0r
    Wr_1[1, 1] = Wr_1[1, 2] = 1               # beta1r
    Wr_1[2, 3], Wr_1[2, 7] = 1, -1            # beta1i
    Wr_1[3, 4], Wr_1[3, 5] = 1, -1            # beta2r
    Wr_1[5, 6] = 1                            # beta3r
    Wr_2 = np.zeros((7, 3))
    Wr_2[4, 0] = Wr_2[4, 1] = 1               # beta2i
    Wr_2[6, 2] = 1                            # beta3i
    return Wc_a, Wc_b, Wr_1, Wr_2


def _block_diag_lhsT(rows_by_block_out, n_in_blocks, blk=16, in_block_of=None,
                     coef=None):
    """lhsT[k_partition, m] for a block-structured map."""
    n_out = len(coef)
    lhsT = np.zeros((n_in_blocks * blk, n_out * blk), dtype=np.float32)
    for mb in range(n_out):
        for kb in range(n_in_blocks):
            if coef[mb][kb] != 0.0:
                for c in range(blk):
                    lhsT[kb * blk + c, mb * blk + c] = coef[mb][kb]
    return lhsT


def _build_consts():
    U, V, A = _form_rows()
    Wc_a, Wc_b, Wr_1, Wr_2 = _combine_mats()

    # form matmuls: input partitions = (16c x 8g), c-major.
    def form_lhsT(rows):
        n_out = rows.shape[0]
        lhsT = np.zeros((128, n_out * 16), dtype=np.float32)
        for j in range(n_out):
            for c in range(16):
                for g in range(G):
                    lhsT[c * G + g, j * 16 + c] = rows[j, g]
        return lhsT

    cU = form_lhsT(U)              # [128, 112]
    cV = form_lhsT(V)              # [128, 112]
    cA = form_lhsT(A)              # [128, 128]
    cCa = _block_diag_lhsT(None, 7, coef=Wc_a).astype(np.float32)   # [112, 128]
    cCb = _block_diag_lhsT(None, 7, coef=Wc_b).astype(np.float32)   # [112, 48]
    cR1 = _block_diag_lhsT(None, 8, coef=Wr_1).astype(np.float32)   # [128, 112]
    cR2 = _block_diag_lhsT(None, 3, coef=Wr_2).astype(np.float32)   # [48, 112]
    return cU, cV, cA, cCa, cCb, cR1, cR2


def _fold_weights(conv_w):
    w = conv_w.reshape(64, C, 16)
    W7 = np.zeros((64, C, 7), dtype=np.float64)
    W7[..., 0] = w[..., 0] + w[..., 7]
    W7[..., 1] = w[..., 1] + w[..., 6]
    W7[..., 2] = w[..., 9] + w[..., 14]
    W7[..., 3] = w[..., 2] + w[..., 5]
    W7[..., 4] = w[..., 10] + w[..., 13]
    W7[..., 5] = w[..., 3] + w[..., 4]
    W7[..., 6] = w[..., 11] + w[..., 12]
    # conv lhsT per q: [112 = (7f x 16c), 64], packed side by side -> [112, 256]
    wf = np.zeros((112, NQ * 64), dtype=np.float32)
    for q in range(NQ):
        for f in range(7):
            for cl in range(16):
                wf[f * 16 + cl, q * 64:(q + 1) * 64] = W7[:, q * 16 + cl, f]
    return wf


def _pack_consts(conv_w):
    """Pack all lhsT constants into one bf16 blob [128, 1008+96].

    layout: cU(112) | cV(112) | cA(128) | cCa(128) | cCb(48) | cR1(112) |
            cR2x(112) | wf(256)   (partition dim padded to 128)
    cR2x is the K=96 extended R2: rows 0-47 = cR2, rows 64-79 = 1/16
    (the +1 ones-trick: t2 carries 16 preset ones partitions at 64:80).
    """
    import ml_dtypes
    cU, cV, cA, cCa, cCb, cR1, cR2 = _build_consts()
    wf = _fold_weights(conv_w.astype(np.float64))
    cR2x = np.zeros((128, 112), np.float32)
    cR2x[0:48] = cR2
    cR2x[64:80] = 1.0 / 16.0

    def pad128(a):
        out = np.zeros((128, a.shape[1]), np.float32)
        out[:a.shape[0]] = a
        return out

    blob = np.concatenate([pad128(cU), pad128(cV), pad128(cA), pad128(cCa),
                           pad128(cCb), pad128(cR1), cR2x, pad128(wf)],
                          axis=1)  # [128, 1008]
    return np.ascontiguousarray(blob).astype(ml_dtypes.bfloat16)


def make_in_maps(x, conv_w, conv_b):
    """Per-core input maps for the program built by _build_program."""
    import ml_dtypes
    x = np.asarray(x)
    B = x.shape[0]
    xr = x.reshape(B, NQ, 128, HWP).astype(ml_dtypes.bfloat16)
    blob = _pack_consts(np.asarray(conv_w))
    bias = np.ascontiguousarray(
        np.asarray(conv_b).astype(np.float32).reshape(64, 1))
    in_maps = []
    for i in range(NCORES):
        in_maps.append(dict(
            x=np.ascontiguousarray(xr[i * BPC:(i + 1) * BPC]),
            cblob=blob, bias=bias))
    return in_maps


_PROG_CACHE = {}


def _build_program(loop_n=1):
    import concourse.bass as bass
    import concourse.bacc as bacc
    import concourse.tile as tile
    import concourse.mybir as mybir

    f32 = mybir.dt.float32
    bf16 = mybir.dt.bfloat16
    nc = bacc.Bacc("TRN2", target_bir_lowering=False, debug=False,
                   num_devices=NCORES)

    x_d = nc.dram_tensor("x", [BPC, NQ, 128, HWP], bf16,
                         kind="ExternalInput").ap()
    cblob_d = nc.dram_tensor("cblob", [128, 1008], bf16,
                             kind="ExternalInput").ap()
    bias_d = nc.dram_tensor("bias", [64, 1], f32, kind="ExternalInput").ap()
    y_d = nc.dram_tensor("y", [BPC, 64, HWP], f32, kind="ExternalOutput").ap()

    LN = mybir.ActivationFunctionType.Ln
    IDENT = mybir.ActivationFunctionType.Identity
    MAX = mybir.AluOpType.max
    MULT = mybir.AluOpType.mult

    with tile.TileContext(nc) as tc:
        with (
            tc.tile_pool(name="consts", bufs=1) as cpool,
            tc.tile_pool(name="xin", bufs=3) as xpool,
            tc.tile_pool(name="sb", bufs=4) as sbpool,
            tc.tile_pool(name="t2p", bufs=3) as t2pool,
            tc.tile_pool(name="rglob", bufs=8) as rpool,
            tc.tile_pool(name="yout", bufs=2) as ypool,
            tc.tile_pool(name="psA", bufs=1, space="PSUM") as psA,
            tc.tile_pool(name="psB", bufs=1, space="PSUM") as psB,
            tc.tile_pool(name="psC", bufs=1, space="PSUM") as psC,
            tc.tile_pool(name="psY", bufs=1, space="PSUM") as psY,
        ):
            # --- load constants (2 DMAs) ---
            cb = cpool.tile([128, 1008], bf16, tag="cblob")
            bias = cpool.tile([64, 1], f32, tag="bias")
            nc.sync.dma_start(out=cb[:], in_=cblob_d)
            nc.sync.dma_start(out=bias[:], in_=bias_d)
            cU = cb[:, 0:112]
            cV = cb[:, 112:224]
            cA = cb[:, 224:352]
            cCa = cb[0:112, 352:480]
            cCb = cb[0:112, 480:528]
            cR1 = cb[0:128, 528:640]
            cR2x = cb[0:96, 640:752]
            wf = cb[0:112, 752:1008]

            # t2 tiles carry 16 preset ones-partitions at [64:80] (the +1
            # bias row of the R2x matmul); [32:64] zeroed so the K=96
            # contraction never multiplies uninitialized SBUF.
            t2bufs = [t2pool.tile([96, S], bf16, tag="t2", name=f"t2i{i}")
                      for i in range(3)]
            for t in t2bufs:
                nc.vector.memset(t[32:64], 0.0)
                nc.vector.memset(t[64:96], 0.0)
                nc.vector.memset(t[64:80], 1.0)

            import contextlib
            loop_cm = (tc.For_i(0, loop_n, 1) if loop_n > 1
                       else contextlib.nullcontext())
            with loop_cm:
              for b in range(BPC):
                  ybig = ypool.tile([64, HWP], f32, tag="ybig")
                  for j in range(NCHUNK):
                      s0 = j * S
                      # one DMA for all 4 q-blocks of this chunk
                      xt4 = xpool.tile([128, NQ, S], bf16, tag="x")
                      nc.sync.dma_start(
                          out=xt4[:],
                          in_=x_d[b, :, :, s0:s0 + S].transpose([1, 0, 2]))
                      rln = [None] * NQ
                      pending = []
                      for q in range(NQ):
                          xt = xt4[:, q]
                          # form matmuls
                          pU = psA.tile([112, S], f32, tag="u")
                          pV = psA.tile([112, S], f32, tag="v")
                          pA = psB.tile([128, S], f32, tag="a")
                          nc.tensor.matmul(pU[:], cU, xt)
                          nc.tensor.matmul(pV[:], cV, xt)
                          nc.tensor.matmul(pA[:], cA, xt)
                          # PSUM -> SBUF evacuations on ACT (only DVE/ACT may
                          # read PSUM; products below keep one PSUM operand)
                          vsb = sbpool.tile([112, S], bf16, tag="vsb")
                          asb = sbpool.tile([128, S], bf16, tag="asb")
                          nc.scalar.copy(vsb[:], pV[:])
                          nc.scalar.copy(asb[:], pA[:])
                          # round-1 products (DVE)
                          m1 = sbpool.tile([112, S], bf16, tag="m1")
                          nc.vector.tensor_mul(m1[:], pU[:], vsb[:])
                          # quadratic combines
                          pCa = psC.tile([128, S], f32, tag="ca")
                          pCb = psC.tile([48, S], f32, tag="cb")
                          nc.tensor.matmul(pCa[:], cCa, m1[:])
                          nc.tensor.matmul(pCb[:], cCb, m1[:])
                          # round-2 products (DVE); t2 writes [0:48] of the
                          # ones-carrying tile
                          t1 = sbpool.tile([128, S], bf16, tag="t1")
                          t2 = t2pool.tile([96, S], bf16, tag="t2")
                          nc.vector.tensor_mul(t1[:], asb[:], pCa[:])
                          nc.vector.tensor_mul(t2[0:48], asb[64:112], pCb[:])
                          # beta combine; R2x ones-rows add +1, so pR = beta+1
                          pR = psB.tile([112, S], f32, tag="rpre", bufs=2)
                          nc.tensor.matmul(pR[:], cR1, t1[:],
                                           start=True, stop=False)
                          nc.tensor.matmul(pR[:], cR2x, t2[:],
                                           start=False, stop=True)
                          # ln(beta+1) straight off PSUM (ACT), then
                          # relu on Pool:  ln(1+relu(b)) == max(ln(b+1), 0).
                          # Emitted one q-step late so ACT never stalls on
                          # this tile's R matmuls.
                          pending.append((pR, q))
                          if len(pending) > 1:
                              pPR, pq = pending.pop(0)
                              lnr = rpool.tile([112, S], bf16, tag="lnr")
                              rl = rpool.tile([112, S], bf16, tag="rl")
                              nc.scalar.activation(lnr[:], pPR[:], LN)
                              nc.gpsimd.tensor_scalar(rl[:], lnr[:], 0.0,
                                                      None, MAX)
                              rln[pq] = rl
                      for pPR, pq in pending:
                          lnr = rpool.tile([112, S], bf16, tag="lnr")
                          rl = rpool.tile([112, S], bf16, tag="rl")
                          nc.scalar.activation(lnr[:], pPR[:], LN)
                          nc.gpsimd.tensor_scalar(rl[:], lnr[:], 0.0, None,
                                                  MAX)
                          rln[pq] = rl
                      # conv
                      pY = psY.tile([64, S], f32, tag="y")
                      for q in range(NQ):
                          nc.tensor.matmul(pY[:],
                                           wf[:, q * 64:(q + 1) * 64],
                                           rln[q][:],
                                           start=(q == 0), stop=(q == NQ - 1))
                      nc.scalar.activation(ybig[:, s0:s0 + S], pY[:], IDENT,
                                           bias=bias[:, 0:1])
                  nc.sync.dma_start(out=y_d[b], in_=ybig[:])
    nc.compile()
    return nc


def kernel(x, conv_w, conv_b):
    from concourse.bass_utils import run_bass_kernel_spmd

    key = "prog"
    if key not in _PROG_CACHE:
        _PROG_CACHE[key] = _build_program()
    nc = _PROG_CACHE[key]

    in_maps = make_in_maps(x, conv_w, conv_b)
    res = run_bass_kernel_spmd(nc, in_maps, core_ids=list(range(NCORES)))
    y = np.concatenate([res.results[i]["y"] for i in range(NCORES)], axis=0)
    return np.ascontiguousarray(
        y.reshape(np.asarray(x).shape[0], 64, 56, 56).astype(np.float32))


# revision 15
# speedup vs baseline: 3.0175x; 3.0175x over previous
"""Trainium2 Bass kernel for nn_BispectrumPool.

Math (validated vs reference):
  F = FFT_8 along the group axis. beta[k] = F1*F[k]*conj(F[1+k mod 8]).
  Due to conjugate symmetry of the real-input FFT:
    beta4=beta3, beta5=beta2, beta6=beta1, beta7=beta0 (real), Im(beta0)=0
  -> only 7 distinct nonzero features per channel:
     [beta0r, beta1r, beta1i, beta2r, beta2i, beta3r, beta3i]
  with
     beta0r = F0*(b1^2+b2^2)
     beta1  = G*conj(F2),  G = F1^2      (Gr=b1^2-b2^2, Gi=2*b1b2)
     beta2  = F2*H,        H = F1*conj(F3)
     beta3  = F4*K,        K = F1*F3
  (b1,b2)=(Re,Im)F1, (b3,b4)=F2, (b5,b6)=F3, b7=F4(real), b0=F0(real).
  feat = ln(1+relu(beta_part)); y = W_folded @ feat + bias, where the 16
  original feature columns fold onto the 7 distinct ones (cols 8,15 drop).

Distribution: pure data parallel, batch 16 -> 2 per core on 8 cores.

Per core pipeline, per (b, 448-col chunk, 16-channel block q):
  DMA x (bf16, one DMA per (b,chunk) covering all 4 q-blocks)
  PE  : U,V,A form matmuls (block-diag DFT rows, bf16 in / f32 PSUM);
        C1a/C1b combine matmuls; Rpre combine matmuls
  Pool: vsb/asb PSUM->SBUF evacuations; t2 product
  DVE : m1 = U*V (7 products); t1 = A*C1a (8 products)
  ACT : relu; ln(1+x); conv bias add
  PE  : 4 conv matmuls (contraction 4x112) -> y [64, 448] -> ybig SBUF
  DMA y once per batch row
"""

import numpy as np

C, G = 64, 8
HWP = 56 * 56            # 3136
S = 512                  # max chunk width; 3136 = 6*512 + 64
CHUNKS = [(i * 512, 512) for i in range(6)] + [(3072, 64)]
NCHUNK = len(CHUNKS)     # 7
NCORES = 8
BPC = 2                  # batches per core
NQ = 4                   # channel blocks of 16


def _form_rows():
    g = np.arange(G)
    B1 = np.cos(2 * np.pi * g / G)
    B2 = -np.sin(2 * np.pi * g / G)
    B3 = np.cos(4 * np.pi * g / G)
    B4 = -np.sin(4 * np.pi * g / G)
    B5 = np.cos(6 * np.pi * g / G)
    B6 = -np.sin(6 * np.pi * g / G)
    B7 = np.cos(np.pi * g)
    B0 = np.ones(G)
    U = np.stack([B1, B2, B1, B1, B2, B2, B1])            # 7 rows
    V = np.stack([B1, B2, B2, B5, B6, B5, B6])            # 7 rows
    # blocks 4..6 = [b3, b4, b7] so the T2 product can slice at partition 64
    # (SBUF engine access must start at a 32-aligned partition)
    A = np.stack([B0, B3, B4, B3, B3, B4, B7, B4])        # 8 rows
    return U, V, A


def _combine_mats():
    # M1 blocks: [b1^2, b2^2, b1b2, b1b5, b2b6, b2b5, b1b6]
    Wc_a = np.zeros((8, 7))
    Wc_a[0, 0] = Wc_a[0, 1] = 1              # S+
    Wc_a[1, 0], Wc_a[1, 1] = 1, -1           # Gr
    Wc_a[2, 2] = 2.0                         # Gi
    Wc_a[3, 2] = 2.0                         # Gi
    Wc_a[4, 3] = Wc_a[4, 4] = 1              # Hr
    Wc_a[5, 5], Wc_a[5, 6] = 1, -1           # Hi
    Wc_a[6, 3], Wc_a[6, 4] = 1, -1           # Kr
    Wc_a[7, 0], Wc_a[7, 1] = 1, -1           # Gr
    Wc_b = np.zeros((3, 7))
    Wc_b[0, 5], Wc_b[0, 6] = 1, -1           # Hi
    Wc_b[1, 3] = Wc_b[1, 4] = 1              # Hr
    Wc_b[2, 5] = Wc_b[2, 6] = 1              # Ki
    # T1 blocks: [b0S+, b3Gr, b4Gi, b3Gi, b3Hr, b4Hi, b7Kr, b4Gr]
    # T2 blocks: [b3Hi, b4Hr, b7Ki]
    Wr_1 = np.zeros((7, 8))
    Wr_1[0, 0] = 1                            # beta0r
    Wr_1[1, 1] = Wr_1[1, 2] = 1               # beta1r
    Wr_1[2, 3], Wr_1[2, 7] = 1, -1            # beta1i
    Wr_1[3, 4], Wr_1[3, 5] = 1, -1            # beta2r
    Wr_1[5, 6] = 1                            # beta3r
    Wr_2 = np.zeros((7, 3))
    Wr_2[4, 0] = Wr_2[4, 1] = 1               # beta2i
    Wr_2[6, 2] = 1                            # beta3i
    return Wc_a, Wc_b, Wr_1, Wr_2


def _block_diag_lhsT(rows_by_block_out, n_in_blocks, blk=16, in_block_of=None,
                     coef=None):
    """lhsT[k_partition, m] for a block-structured map."""
    n_out = len(coef)
    lhsT = np.zeros((n_in_blocks * blk, n_out * blk), dtype=np.float32)
    for mb in range(n_out):
        for kb in range(n_in_blocks):
            if coef[mb][kb] != 0.0:
                for c in range(blk):
                    lhsT[kb * blk + c, mb * blk + c] = coef[mb][kb]
    return lhsT


def _build_consts():
    U, V, A = _form_rows()
    Wc_a, Wc_b, Wr_1, Wr_2 = _combine_mats()

    # form matmuls: input partitions = (16c x 8g), c-major.
    def form_lhsT(rows):
        n_out = rows.shape[0]
        lhsT = np.zeros((128, n_out * 16), dtype=np.float32)
        for j in range(n_out):
            for c in range(16):
                for g in range(G):
                    lhsT[c * G + g, j * 16 + c] = rows[j, g]
        return lhsT

    cU = form_lhsT(U)              # [128, 112]
    cV = form_lhsT(V)              # [128, 112]
    cA = form_lhsT(A)              # [128, 128]
    cCa = _block_diag_lhsT(None, 7, coef=Wc_a).astype(np.float32)   # [112, 128]
    cCb = _block_diag_lhsT(None, 7, coef=Wc_b).astype(np.float32)   # [112, 48]
    cR1 = _block_diag_lhsT(None, 8, coef=Wr_1).astype(np.float32)   # [128, 112]
    cR2 = _block_diag_lhsT(None, 3, coef=Wr_2).astype(np.float32)   # [48, 112]
    return cU, cV, cA, cCa, cCb, cR1, cR2


def _fold_weights(conv_w):
    w = conv_w.reshape(64, C, 16)
    W7 = np.zeros((64, C, 7), dtype=np.float64)
    W7[..., 0] = w[..., 0] + w[..., 7]
    W7[..., 1] = w[..., 1] + w[..., 6]
    W7[..., 2] = w[..., 9] + w[..., 14]
    W7[..., 3] = w[..., 2] + w[..., 5]
    W7[..., 4] = w[..., 10] + w[..., 13]
    W7[..., 5] = w[..., 3] + w[..., 4]
    W7[..., 6] = w[..., 11] + w[..., 12]
    # conv lhsT per q: [112 = (7f x 16c), 64], packed side by side -> [112, 256]
    wf = np.zeros((112, NQ * 64), dtype=np.float32)
    for q in range(NQ):
        for f in range(7):
            for cl in range(16):
                wf[f * 16 + cl, q * 64:(q + 1) * 64] = W7[:, q * 16 + cl, f]
    return wf


def _pack_consts(conv_w):
    """Pack all lhsT constants into one bf16 blob [128, 1008+96].

    layout: cU(112) | cV(112) | cA(128) | cCa(128) | cCb(48) | cR1(112) |
            cR2x(112) | wf(256)   (partition dim padded to 128)
    cR2x is the K=96 extended R2: rows 0-47 = cR2, rows 64-79 = 1/16
    (the +1 ones-trick: t2 carries 16 preset ones partitions at 64:80).
    """
    import ml_dtypes
    cU, cV, cA, cCa, cCb, cR1, cR2 = _build_consts()
    wf = _fold_weights(conv_w.astype(np.float64))
    cR2x = np.zeros((128, 112), np.float32)
    cR2x[0:48] = cR2

    def pad128(a):
        out = np.zeros((128, a.shape[1]), np.float32)
        out[:a.shape[0]] = a
        return out

    blob = np.concatenate([pad128(cU), pad128(cV), pad128(cA), pad128(cCa),
                           pad128(cCb), pad128(cR1), cR2x, pad128(wf)],
                          axis=1)  # [128, 1008]
    return (np.ascontiguousarray(blob).astype(np.float32),
            np.ascontiguousarray(wf).astype(np.float32))


def make_in_maps(x, conv_w, conv_b):
    """Per-core input maps for the program built by _build_program."""
    import ml_dtypes
    x = np.asarray(x)
    B = x.shape[0]
    xr = x.reshape(B, NQ, 128, HWP).astype(np.float32)
    blob, cwf = _pack_consts(np.asarray(conv_w))
    bias = np.ascontiguousarray(
        np.asarray(conv_b).astype(np.float32).reshape(64, 1))
    in_maps = []
    for i in range(NCORES):
        in_maps.append(dict(
            x=np.ascontiguousarray(xr[i * BPC:(i + 1) * BPC]),
            cblob=blob, cwf=cwf, bias=bias))
    return in_maps


_PROG_CACHE = {}


def _build_program(loop_n=1):
    import concourse.bass as bass
    import concourse.bacc as bacc
    import concourse.tile as tile
    import concourse.mybir as mybir

    f32 = mybir.dt.float32
    f32r = mybir.dt.float32r
    bf16 = mybir.dt.bfloat16
    nc = bacc.Bacc("TRN2", target_bir_lowering=False, debug=False,
                   num_devices=NCORES)

    x_d = nc.dram_tensor("x", [BPC, NQ, 128, HWP], f32r,
                         kind="ExternalInput").ap()
    cblob_d = nc.dram_tensor("cblob", [128, 1008], f32r,
                             kind="ExternalInput").ap()
    cwf_d = nc.dram_tensor("cwf", [112, 256], f32r,
                           kind="ExternalInput").ap()
    bias_d = nc.dram_tensor("bias", [64, 1], f32, kind="ExternalInput").ap()
    y_d = nc.dram_tensor("y", [BPC, 64, HWP], f32, kind="ExternalOutput").ap()

    LN = mybir.ActivationFunctionType.Ln
    IDENT = mybir.ActivationFunctionType.Identity
    MAX = mybir.AluOpType.max
    MULT = mybir.AluOpType.mult

    with tile.TileContext(nc) as tc:
        with (
            tc.tile_pool(name="consts", bufs=1) as cpool,
            tc.tile_pool(name="xin", bufs=3) as xpool,
            tc.tile_pool(name="sb", bufs=4) as sbpool,
            tc.tile_pool(name="t2p", bufs=3) as t2pool,
            tc.tile_pool(name="rglob", bufs=8) as rpool,
            tc.tile_pool(name="yout", bufs=2) as ypool,
            tc.tile_pool(name="psA", bufs=1, space="PSUM") as psA,
            tc.tile_pool(name="psB", bufs=1, space="PSUM") as psB,
            tc.tile_pool(name="psC", bufs=1, space="PSUM") as psC,
            tc.tile_pool(name="psY", bufs=1, space="PSUM") as psY,
        ):
            # --- load constants (2 DMAs) ---
            cb = cpool.tile([128, 1008], f32r, tag="cblob")
            cwf = cpool.tile([112, 256], f32r, tag="cwf")
            bias = cpool.tile([64, 1], f32, tag="bias")
            nc.sync.dma_start(out=cb[:], in_=cblob_d)
            nc.sync.dma_start(out=cwf[:], in_=cwf_d)
            nc.sync.dma_start(out=bias[:], in_=bias_d)
            cU = cb[:, 0:112]
            cV = cb[:, 112:224]
            cA = cb[:, 224:352]
            cCa = cb[0:112, 352:480]
            cCb = cb[0:112, 480:528]
            cR1 = cb[0:128, 528:640]
            cR2x = cb[0:48, 640:752]
            wf = cwf


            import contextlib
            loop_cm = (tc.For_i(0, loop_n, 1) if loop_n > 1
                       else contextlib.nullcontext())
            with loop_cm:
              for b in range(BPC):
                  ybig = ypool.tile([64, HWP], f32, tag="ybig")
                  for (s0, W) in CHUNKS:
                      # one DMA for all 4 q-blocks of this chunk
                      xt4 = xpool.tile([128, NQ, S], f32r, tag="x")
                      nc.sync.dma_start(
                          out=xt4[:, :, 0:W],
                          in_=x_d[b, :, :, s0:s0 + W].transpose([1, 0, 2]))
                      pYB = psY.tile([64, S], f32, tag="ycb")
                      rln = [None] * NQ
                      pending = []
                      for q in range(NQ):
                          xt = xt4[:, q, 0:W]
                          # form matmuls
                          pU = psA.tile([112, S], f32, tag="u")
                          pV = psA.tile([112, S], f32, tag="v")
                          pA = psB.tile([128, S], f32, tag="a")
                          nc.tensor.matmul(pU[:, 0:W], cU, xt)
                          nc.tensor.matmul(pV[:, 0:W], cV, xt)
                          nc.tensor.matmul(pA[:, 0:W], cA, xt)
                          # PSUM -> SBUF evacuations on ACT (only DVE/ACT may
                          # read PSUM; products below keep one PSUM operand)
                          vsb = sbpool.tile([112, S], f32r, tag="vsb")
                          asb = sbpool.tile([128, S], f32r, tag="asb")
                          nc.scalar.copy(vsb[:, 0:W], pV[:, 0:W])
                          nc.scalar.copy(asb[:, 0:W], pA[:, 0:W])
                          # round-1 products (DVE)
                          m1 = sbpool.tile([112, S], f32r, tag="m1")
                          nc.vector.tensor_mul(m1[:, 0:W], pU[:, 0:W],
                                               vsb[:, 0:W])
                          # quadratic combines; Cb lands in the Y bank's
                          # spare partitions (64:112)
                          pCa = psC.tile([128, S], f32, tag="ca", bufs=2)
                          pCb = psC.tile([48, S], f32, tag="cb")
                          nc.tensor.matmul(pCa[:, 0:W], cCa, m1[:, 0:W])
                          nc.tensor.matmul(pCb[:, 0:W], cCb, m1[:, 0:W])
                          # round-2 products (DVE); t2 writes [0:48] of the
                          # ones-carrying tile
                          t1 = sbpool.tile([128, S], f32r, tag="t1")
                          t2 = t2pool.tile([48, S], f32r, tag="t2")
                          nc.vector.tensor_mul(t1[:, 0:W], asb[:, 0:W],
                                               pCa[:, 0:W])
                          nc.vector.tensor_mul(t2[:, 0:W], asb[64:112, 0:W],
                                               pCb[:, 0:W])
                          # beta combine; R2x ones-rows add +1, so pR = beta+1
                          pR = psB.tile([112, S], f32, tag="rpre")
                          nc.tensor.matmul(pR[:, 0:W], cR1, t1[:, 0:W],
                                           start=True, stop=False)
                          nc.tensor.matmul(pR[:, 0:W], cR2x, t2[:, 0:W],
                                           start=False, stop=True)
                          # relu(+1) then ln(1+x), alternating DVE/ACT
                          rg = rpool.tile([112, S], f32, tag="rg")
                          rl = rpool.tile([112, S], f32r, tag="rl")
                          ADD = mybir.AluOpType.add
                          if q % 2 == 0:
                              nc.vector.tensor_scalar(rg[:, 0:W], pR[:, 0:W],
                                                      0.0, 1.0, MAX, ADD)
                              nc.scalar.activation(rl[:, 0:W], rg[:, 0:W], LN)
                          else:
                              nc.scalar.activation(
                                  rg[:, 0:W], pR[:, 0:W],
                                  mybir.ActivationFunctionType.Relu)
                              nc.scalar.activation(rl[:, 0:W], rg[:, 0:W], LN,
                                                   bias=1.0)
                          rln[q] = rl
                      # conv
                      for q in range(NQ):
                          nc.tensor.matmul(pYB[:, 0:W],
                                           wf[:, q * 64:(q + 1) * 64],
                                           rln[q][:, 0:W],
                                           start=(q == 0), stop=(q == NQ - 1))
                      nc.scalar.activation(ybig[:, s0:s0 + W], pYB[:, 0:W],
                                           IDENT, bias=bias[:, 0:1])
                  nc.sync.dma_start(out=y_d[b], in_=ybig[:])
    nc.compile()
    return nc


def kernel(x, conv_w, conv_b):
    from concourse.bass_utils import run_bass_kernel_spmd

    key = "prog"
    if key not in _PROG_CACHE:
        _PROG_CACHE[key] = _build_program()
    nc = _PROG_CACHE[key]

    in_maps = make_in_maps(x, conv_w, conv_b)
    res = run_bass_kernel_spmd(nc, in_maps, core_ids=list(range(NCORES)))
    y = np.concatenate([res.results[i]["y"] for i in range(NCORES)], axis=0)
    return np.ascontiguousarray(
        y.reshape(np.asarray(x).shape[0], 64, 56, 56).astype(np.float32))
